# revision 64
# baseline (speedup 1.0000x reference)
"""Trainium2 Bass kernel for 8-head self-attention (nn_Attention2).

Sharding: one head per NeuronCore (tensor parallel over heads).
Each core computes, for its head h (d = 128 = partition width, C = 4096):
    K^T = Wk_h^T x^T           [d, C]
    Q^T = Wq_h^T x^T           [d, C]
    V   = x Wv_h               [C, d]   (128-row k-tiles)
    S^T tile = K_tile Q_chunk^T         (scores, key-major layout)
    P = exp(S^T / sqrt(d))              (fp16; no max-sub: |s~| < 6 here)
    O^T += V_tile^T P          [d, 512] per q-chunk, PSUM accum
    den[q] = P^T ones                   (1-column matmuls -> q-partition layout)
    partial = (O^T)^T Wp_h * (1/den)    (normalization fused into the
                                         projection-output copy as a
                                         per-partition scale)
The host sums the 8 per-head partials (tensor-parallel all-reduce) and adds
the bias.  All matmuls run in fp16 with fp32 PSUM accumulate.

Schedule: x^T streams in column-blocks of 512; per block PE computes the
K^T/Q^T/V contributions so compute starts ~3.5us into the DMA (warmup
matmuls arm the PE p-state ramp during the DMA window).  Score tiles for
q-chunk qc+1 are emitted during qc's flash-attention window so the
activation engine (the exp stream, ~133us) runs a full chunk ahead of the
PE's P*V consumption and never gates it.  Q^T blocks 2-7 and V blocks 6-7
are deferred into the windows as PE filler.

PSUM note: a matmul with start=True zeroes its whole 2KB bank ("zero
region") and only one accumulation group may be pending per region, so the
four per-window den columns share one group: only the first den matmul
starts it (the bank-zero clears all four columns), only the last stops it.
"""

import os

import numpy as np

C = 4096
G = 1024
D = 128
NCORES = 8
SCALE = float(D) ** -0.5
N_WARMUP = int(os.environ.get("N_WARMUP", "6"))

_CACHE = {}


def _build():
    import concourse.bacc as bacc
    import concourse.mybir as mybir
    from concourse.tile import TileContext

    F16 = mybir.dt.float16
    F32 = mybir.dt.float32
    Exp = mybir.ActivationFunctionType.Exp

    KC = G // 128   # 8 contraction chunks over the model dim
    NB = C // 512   # 8 column blocks of x^T (= q-chunks = key blocks)
    NQ = NB
    NT = C // 256   # 16 t-steps per q-chunk (2 key tiles each)

    nc = bacc.Bacc("TRN2", target_bir_lowering=False, debug=False,
                   num_devices=NCORES)
    # All inputs are host-prearranged SBUF images (partition-major).
    xtb_d = nc.dram_tensor("xtb", [128, NB * KC * 512], F16,
                           kind="ExternalInput").ap()
    wkb_d = nc.dram_tensor("wkb", [128, KC * D], F16, kind="ExternalInput").ap()
    wqb_d = nc.dram_tensor("wqb", [128, KC * D], F16, kind="ExternalInput").ap()
    wvb_d = nc.dram_tensor("wvb", [128, KC * D], F16, kind="ExternalInput").ap()
    wpb_d = nc.dram_tensor("wpb", [D, G], F16, kind="ExternalInput").ap()
    out_d = nc.dram_tensor("partial", [C, G], F16, kind="ExternalOutput").ap()

    with TileContext(nc) as tc:
        with (
            tc.tile_pool(name="persist", bufs=1) as big,
            tc.tile_pool(name="pt", bufs=20) as pt_pool,
            tc.tile_pool(name="recip", bufs=2) as recip_pool,
            tc.tile_pool(name="ob", bufs=4) as ob_pool,
            tc.tile_pool(name="ps_st", bufs=2, space="PSUM") as st_pool,
        ):
            xt_sb = big.tile([128, NB * KC * 512], F16)  # col j*4096+g*512+c
            wk_sb = big.tile([128, KC * D], F16)
            wq_sb = big.tile([128, KC * D], F16)
            wv_sb = big.tile([128, KC * D], F16)
            wp_sb = big.tile([128, G], F16)
            kt_sb = big.tile([128, C], F16)              # K^T
            qt_sb = big.tile([128, C], F16)              # Q^T
            v_sb = big.tile([128, C], F16)               # V, k-tile c at c*128
            ot_sb = big.tile([128, C], F16)              # O^T (unnormalized)
            ones_sb = big.tile([128, 1], F16)

            nc.vector.memset(ones_sb[:], 1.0)

            # ---- input DMA issues, split across three DGE paths ----
            # SP (HWDGE): the x^T block-0 quarters (the critical path).
            for q in range(4):
                nc.sync.dma_start(xt_sb[:, q * 1024:(q + 1) * 1024],
                                  xtb_d[:, q * 1024:(q + 1) * 1024])
            # Pool (SWDGE): wk halves, in parallel with SP's issues.
            nc.gpsimd.dma_start(wk_sb[:, 0:512], wkb_d[:, 0:512])
            nc.gpsimd.dma_start(wk_sb[:, 512:1024], wkb_d[:, 512:1024])
            # Act (HWDGE): wq halves, x^T block 1 quartered (it gates the
            # second kt block), wv, then the remaining blocks whole.
            nc.scalar.dma_start(wq_sb[:, 0:512], wqb_d[:, 0:512])
            nc.scalar.dma_start(xt_sb[:, 4096:4096 + 1024],
                                xtb_d[:, 4096:4096 + 1024])
            nc.scalar.dma_start(wq_sb[:, 512:1024], wqb_d[:, 512:1024])
            for q in range(1, 4):
                nc.scalar.dma_start(
                    xt_sb[:, 4096 + q * 1024:4096 + (q + 1) * 1024],
                    xtb_d[:, 4096 + q * 1024:4096 + (q + 1) * 1024])
            nc.scalar.dma_start(wv_sb[:], wvb_d[:, :])
            nc.scalar.dma_start(xt_sb[:, 2 * 4096:2 * 4096 + 2048],
                                xtb_d[:, 2 * 4096:2 * 4096 + 2048])
            nc.scalar.dma_start(xt_sb[:, 2 * 4096 + 2048:3 * 4096],
                                xtb_d[:, 2 * 4096 + 2048:3 * 4096])
            for j in range(3, NB):
                nc.scalar.dma_start(xt_sb[:, j * 4096:(j + 1) * 4096],
                                    xtb_d[:, j * 4096:(j + 1) * 4096])
            nc.scalar.dma_start(wp_sb[:], wpb_d[:, :])

            pts = {}

            def emit_st(qc, t):
                q_sl = qt_sb[:, qc * 512:(qc + 1) * 512]
                st = st_pool.tile([128, 1024], F32, tag="st", name="st")
                nc.tensor.matmul(st[:, 0:512],
                                 kt_sb[:, (2 * t) * 128:(2 * t + 1) * 128],
                                 q_sl, start=True, stop=True)
                nc.tensor.matmul(st[:, 512:1024],
                                 kt_sb[:, (2 * t + 1) * 128:(2 * t + 2) * 128],
                                 q_sl, start=True, stop=True)
                pt = pt_pool.tile([128, 1024], F16, tag="pt", name="pt")
                nc.scalar.activation(pt[:], st[:], Exp, scale=SCALE)
                pts[(qc, t)] = pt

            def emit_st_half(qh, t):
                # score step for a 256-query half-window: 4 k-tiles per tile
                # so the exp instruction stays at the efficient 1024 width.
                # Each 2KB zero region spans two 256-col segments: start on
                # the first, stop on the second.
                q_sl = qt_sb[:, qh * 256:(qh + 1) * 256]
                st = st_pool.tile([128, 1024], F32, tag="st", name="sth")
                for s in range(4):
                    kk = 4 * t + s
                    nc.tensor.matmul(st[:, s * 256:(s + 1) * 256],
                                     kt_sb[:, kk * 128:(kk + 1) * 128],
                                     q_sl, start=(s % 2 == 0),
                                     stop=(s % 2 == 1))
                pt = pt_pool.tile([128, 1024], F16, tag="pt", name="pth")
                nc.scalar.activation(pt[:], st[:], Exp, scale=SCALE)
                pts[("h", qh, t)] = pt

            def emit_qt_block(pool, j, tag="acc"):
                a = pool.tile([128, 512], F32, tag=tag, name="qta")
                for g in range(KC):
                    nc.tensor.matmul(
                        a[:], wq_sb[:, g * 128:(g + 1) * 128],
                        xt_sb[:, j * 4096 + g * 512:j * 4096 + (g + 1) * 512],
                        start=(g == 0), stop=(g == KC - 1))
                nc.vector.tensor_copy(qt_sb[:, j * 512:(j + 1) * 512], a[:])

            def emit_v_block(pool, j, tag="acc"):
                a = pool.tile([128, 512], F32, tag=tag, name="va")
                for i in range(4):
                    for g in range(KC):
                        nc.tensor.matmul(
                            a[:, i * 128:(i + 1) * 128],
                            xt_sb[:, j * 4096 + g * 512 + i * 128:
                                  j * 4096 + g * 512 + (i + 1) * 128],
                            wv_sb[:, g * 128:(g + 1) * 128],
                            start=(g == 0), stop=(g == KC - 1))
                nc.vector.tensor_copy(v_sb[:, j * 512:(j + 1) * 512], a[:])

            # ---- phase 1: K^T (all), Q^T 0-1, V 0-5, qc0 scores ----
            with tc.tile_pool(name="ps_p1", bufs=4, space="PSUM") as ph1:
                # warmup: arm the PE p-state ramp while the first DMAs land
                # (rhs is a stride-0 broadcast of the ones column so the
                # chain only waits the tiny ones memset)
                warm_rhs = ones_sb[:, 0:1].broadcast_to([128, 512])
                wps = ph1.tile([128, 512], F32, tag="acc", name="wps")
                for w in range(N_WARMUP):
                    nc.tensor.matmul(wps[0:1, :], ones_sb[:, 0:1], warm_rhs,
                                     start=(w == 0), stop=(w == N_WARMUP - 1))

                # block 0: kt/qt interleaved per arriving x^T quarter
                ka = ph1.tile([128, 512], F32, tag="acc", name="kta")
                qa = ph1.tile([128, 512], F32, tag="acc", name="qta")
                for q in range(4):
                    for g in (2 * q, 2 * q + 1):
                        nc.tensor.matmul(
                            ka[:], wk_sb[:, g * 128:(g + 1) * 128],
                            xt_sb[:, g * 512:(g + 1) * 512],
                            start=(g == 0), stop=(g == KC - 1))
                    for g in (2 * q, 2 * q + 1):
                        nc.tensor.matmul(
                            qa[:], wq_sb[:, g * 128:(g + 1) * 128],
                            xt_sb[:, g * 512:(g + 1) * 512],
                            start=(g == 0), stop=(g == KC - 1))
                nc.vector.tensor_copy(kt_sb[:, 0:512], ka[:])
                nc.vector.tensor_copy(qt_sb[:, 0:512], qa[:])
                emit_st(0, 0)
                emit_st(0, 1)

                for j in range(1, NB):
                    if j == NB - 1:
                        # the last V block before the last kt block so its
                        # PSUM->SBUF copy drains while PE runs the final
                        # score matmuls (the ph1 pool teardown gates the
                        # first window otherwise)
                        emit_v_block(ph1, j - 2)
                    a = ph1.tile([128, 512], F32, tag="acc", name="kta")
                    for g in range(KC):
                        nc.tensor.matmul(
                            a[:], wk_sb[:, g * 128:(g + 1) * 128],
                            xt_sb[:, j * 4096 + g * 512:j * 4096 + (g + 1) * 512],
                            start=(g == 0), stop=(g == KC - 1))
                    nc.vector.tensor_copy(kt_sb[:, j * 512:(j + 1) * 512], a[:])
                    emit_st(0, 2 * j)
                    emit_st(0, 2 * j + 1)
                    if j == 1:
                        emit_qt_block(ph1, 1)
                    if 2 <= j < NB - 1:
                        emit_v_block(ph1, j - 2)

            # ---- phase 2: attention windows + projection ----
            recips = {}
            with (
                tc.tile_pool(name="ps_o", bufs=3, space="PSUM") as o_pool,
                tc.tile_pool(name="ps_den", bufs=1, space="PSUM") as den_pool,
            ):
                # One den group per window in a single bank: the first matmul's
                # start=True zeroes the whole bank (all 4 columns); re-start in
                # the next window is WAR-ordered behind the reciprocal read.
                den_ps = den_pool.tile([128, 4], F32)

                def emit_proj(pqc, jj):
                    cq = pqc * 4 + jj
                    lhs = ot_sb[:, cq * 128:(cq + 1) * 128]
                    rs = recips[pqc][:, jj:jj + 1]
                    ob = ob_pool.tile([128, 1024], F16, name="ob")
                    pa = o_pool.tile([128, 512], F32, tag="o", name="pa")
                    nc.tensor.matmul(pa[:], lhs, wp_sb[:, 0:512],
                                     start=True, stop=True)
                    nc.vector.tensor_scalar_mul(ob[:, 0:512], pa[:], rs)
                    pb = o_pool.tile([128, 512], F32, tag="o", name="pb")
                    nc.tensor.matmul(pb[:], lhs, wp_sb[:, 512:1024],
                                     start=True, stop=True)
                    nc.vector.tensor_scalar_mul(ob[:, 512:1024], pb[:], rs)
                    nc.sync.dma_start(out_d[cq * 128:(cq + 1) * 128, :], ob[:])

                for qc in range(NQ - 1):
                    o_ps = o_pool.tile([128, 512], F32, tag="o", name="o_ps")
                    for t in range(NT):
                        if qc < NQ - 2:
                            emit_st(qc + 1, t)
                        elif t % 2 == 0:
                            # half-window A's scores (4 k-tiles per step)
                            emit_st_half(2 * NQ - 2, t // 2)
                        pt = pts.pop((qc, t))
                        nc.tensor.matmul(o_ps[:],
                                         v_sb[:, (2 * t) * 128:(2 * t + 1) * 128],
                                         pt[:, 0:512],
                                         start=(t == 0), stop=False)
                        nc.tensor.matmul(o_ps[:],
                                         v_sb[:, (2 * t + 1) * 128:(2 * t + 2) * 128],
                                         pt[:, 512:1024],
                                         start=False, stop=(t == NT - 1))
                        for hh in range(2):
                            for qq in range(4):
                                nc.tensor.matmul(
                                    den_ps[:, qq:qq + 1],
                                    pt[:, hh * 512 + qq * 128:
                                       hh * 512 + (qq + 1) * 128],
                                    ones_sb[:],
                                    start=(t == 0 and hh == 0 and qq == 0),
                                    stop=(t == NT - 1 and hh == 1 and qq == 3))
                        if qc > 0 and t % 4 == 2:
                            # one t-step before the multiple-of-4 boundary so
                            # the last pp-copy drains before the next window's
                            # o_ps allocation needs its slot
                            emit_proj(qc - 1, (t - 2) // 4)
                        # deferred phase-1 work as PE filler (PSUM slots
                        # borrowed from the o/pp pool)
                        if qc == 0:
                            if t == 4:
                                emit_v_block(o_pool, 6, tag="o")
                            elif t == 8:
                                emit_v_block(o_pool, 7, tag="o")
                            elif t == 12:
                                emit_qt_block(o_pool, 2, tag="o")
                        elif qc <= 5 and t == 8:
                            emit_qt_block(o_pool, qc + 2, tag="o")
                    recip = recip_pool.tile([128, 4], F32, name="recip")
                    nc.vector.reciprocal(recip[:], den_ps[:])
                    recips[qc] = recip
                    nc.vector.tensor_copy(
                        ot_sb[:, qc * 512:(qc + 1) * 512], o_ps[:])

                # ---- half-windows A and B (256 queries each) ----
                # A's projection and output DMA drain during B, so only two
                # row-tiles remain after the final P*V.
                def emit_proj_h(cq, rs, pa, pb, split=False, last=False):
                    lhs = ot_sb[:, cq * 128:(cq + 1) * 128]
                    ob = ob_pool.tile([128, 1024], F16, name="obh")
                    nc.tensor.matmul(pa, lhs, wp_sb[:, 0:512],
                                     start=True, stop=True)
                    nc.tensor.matmul(pb, lhs, wp_sb[:, 512:1024],
                                     start=True, stop=True)
                    if last:
                        # both halves on DVE (its queue flows without the
                        # cross-engine sem hop Act would pay) + per-half DMAs
                        nc.vector.tensor_scalar_mul(ob[:, 0:512], pa, rs)
                        nc.sync.dma_start(out_d[cq * 128:(cq + 1) * 128, 0:512],
                                          ob[:, 0:512])
                        nc.vector.tensor_scalar_mul(ob[:, 512:1024], pb, rs)
                        nc.sync.dma_start(
                            out_d[cq * 128:(cq + 1) * 128, 512:1024],
                            ob[:, 512:1024])
                    elif split:
                        # normalization copies split across DVE and Act so
                        # the endgame drains on two engines in parallel
                        nc.vector.tensor_scalar_mul(ob[:, 0:512], pa, rs)
                        nc.scalar.mul(ob[:, 512:1024], pb, rs)
                        nc.sync.dma_start(out_d[cq * 128:(cq + 1) * 128, :],
                                          ob[:])
                    else:
                        nc.vector.tensor_scalar_mul(ob[:, 0:512], pa, rs)
                        nc.vector.tensor_scalar_mul(ob[:, 512:1024], pb, rs)
                        nc.sync.dma_start(out_d[cq * 128:(cq + 1) * 128, :],
                                          ob[:])

                def emit_st_q(qh, t):
                    # quarter-window score step: 8 k-tiles x 128 queries
                    q_sl = qt_sb[:, qh * 128:(qh + 1) * 128]
                    st = st_pool.tile([128, 1024], F32, tag="st", name="stq")
                    for s in range(8):
                        kk = 8 * t + s
                        nc.tensor.matmul(st[:, s * 128:(s + 1) * 128],
                                         kt_sb[:, kk * 128:(kk + 1) * 128],
                                         q_sl, start=(s % 4 == 0),
                                         stop=(s % 4 == 3))
                    pt = pt_pool.tile([128, 1024], F16, tag="pt", name="ptq")
                    nc.scalar.activation(pt[:], st[:], Exp, scale=SCALE)
                    pts[("q", qh, t)] = pt

                # half-window A (queries 3584-3839): emits the first
                # quarter-window's scores and window 6's projection
                o_psA = o_pool.tile([128, 256], F32, tag="o", name="o_psA")
                for t in range(8):
                    if t % 2 == 0:
                        emit_st_q(2 * C // 256 - 2, t // 2)
                    pt = pts.pop(("h", 2 * NQ - 2, t))
                    for s in range(4):
                        kk = 4 * t + s
                        nc.tensor.matmul(
                            o_psA[:], v_sb[:, kk * 128:(kk + 1) * 128],
                            pt[:, s * 256:(s + 1) * 256],
                            start=(t == 0 and s == 0),
                            stop=(t == 7 and s == 3))
                        for qq in range(2):
                            nc.tensor.matmul(
                                den_ps[:, qq:qq + 1],
                                pt[:, s * 256 + qq * 128:
                                   s * 256 + (qq + 1) * 128],
                                ones_sb[:],
                                start=(t == 0 and s == 0 and qq == 0),
                                stop=(t == 7 and s == 3 and qq == 1))
                    if t in (1, 3, 5):
                        emit_proj(NQ - 2, (t - 1) // 2)
                    elif t == 6:
                        # last pair one step early so its copies drain before
                        # the A->B1 boundary needs the DVE
                        emit_proj(NQ - 2, 3)
                nc.vector.tensor_copy(
                    ot_sb[:, (NQ - 1) * 512:(NQ - 1) * 512 + 256], o_psA[:])
                recipA = recip_pool.tile([128, 2], F32, name="recipA")
                nc.vector.reciprocal(recipA[:], den_ps[:, 0:2])
                recips["A"] = recipA

                def quarter_window(qi, next_scores, projs):
                    # qi: quarter index 30/31 (128 queries); 4 steps of 8
                    # k-tiles; projs: list of (t, cq, recip_col_ap).
                    # den matmuls run one t-step late so the den-bank restart
                    # (WAR on the previous quarter's reciprocal) never stalls
                    # the PE.
                    o_psq = o_pool.tile([128, 128], F32, tag="o", name="o_psq")
                    live = {}

                    def emit_den(dt_, first, final):
                        pt = live[dt_]
                        for s in range(8):
                            nc.tensor.matmul(
                                den_ps[:, 0:1], pt[:, s * 128:(s + 1) * 128],
                                ones_sb[:],
                                start=(first and s == 0),
                                stop=(final and s == 7))

                    for t in range(4):
                        if next_scores:
                            emit_st_q(qi + 1, t)
                        pt = pts.pop(("q", qi, t))
                        live[t] = pt
                        for s in range(8):
                            kk = 8 * t + s
                            nc.tensor.matmul(
                                o_psq[:], v_sb[:, kk * 128:(kk + 1) * 128],
                                pt[:, s * 128:(s + 1) * 128],
                                start=(t == 0 and s == 0),
                                stop=(t == 3 and s == 7))
                        if t >= 2:
                            emit_den(t - 2, first=(t == 2), final=False)
                        for (pt_t, cq, rs) in projs:
                            if t == pt_t:
                                pa = o_pool.tile([128, 512], F32, tag="o",
                                                 name="pqa")
                                pb = o_pool.tile([128, 512], F32, tag="o",
                                                 name="pqb")
                                emit_proj_h(cq, rs, pa[:], pb[:], split=True)
                    emit_den(2, first=False, final=False)
                    emit_den(3, first=False, final=True)
                    return o_psq

                # quarter B1 (queries 3840-3967): emits B2's scores and A's
                # projection
                o_psB1 = quarter_window(
                    2 * C // 256 - 2, True,
                    [(1, (NQ - 1) * 4, recipA[:, 0:1]),
                     (2, (NQ - 1) * 4 + 1, recipA[:, 1:2])])
                nc.vector.tensor_copy(ot_sb[:, C - 256:C - 128], o_psB1[:])
                recipB1 = recip_pool.tile([128, 1], F32, name="recipB1")
                nc.vector.reciprocal(recipB1[:], den_ps[:, 0:1])

                # quarter B2 (queries 3968-4095): B1's projection inside
                o_psB2 = quarter_window(
                    2 * C // 256 - 1, False,
                    [(2, (NQ - 1) * 4 + 2, recipB1[:, 0:1])])
                nc.vector.tensor_copy(ot_sb[:, C - 128:C], o_psB2[:])
                recipB2 = recip_pool.tile([128, 1], F32, name="recipB2")
                nc.vector.reciprocal(recipB2[:], den_ps[:, 0:1])

                # ---- tail: only the final row-tile's projection remains ----
                pp = st_pool.tile([128, 1024], F32, tag="st", name="ppt")
                emit_proj_h((NQ - 1) * 4 + 3, recipB2[:, 0:1],
                            pp[:, 0:512], pp[:, 512:1024], last=True)

    nc.compile()
    return nc


def _get_nc():
    if "nc" not in _CACHE:
        _CACHE["nc"] = _build()
    return _CACHE["nc"]


def _install_neff_cache():
    """Content-hash cache for the walrus NEFF compile (~5 min saved on
    repeat runs of the same kernel)."""
    if _CACHE.get("neff_cache"):
        return
    import hashlib
    import shutil
    import concourse.bass_utils as bu
    import concourse.bass2jax as b2j

    orig = bu.compile_bir_kernel
    # The BIR embeds source paths/lines (debug info), so hashing it would
    # miss the cache when this file runs from a different directory. The
    # kernel is fully determined by this file's source, so key on that.
    with open(__file__, "rb") as f:
        src_hash = hashlib.sha256(f.read()).hexdigest()[:32]

    def cached_compile(bir_json, tmpdir, neff_name="file.neff"):
        key = src_hash
        cdir = os.path.expanduser("~/.cache/bass_neff")
        os.makedirs(cdir, exist_ok=True)
        cpath = os.path.join(cdir, key + ".neff")
        dst = os.path.join(tmpdir, neff_name)
        if os.path.exists(cpath):
            shutil.copy(cpath, dst)
            return dst
        out = orig(bir_json, tmpdir, neff_name)
        try:
            shutil.copy(out, cpath)
        except OSError:
            pass
        return out

    bu.compile_bir_kernel = cached_compile
    b2j.compile_bir_kernel = cached_compile
    _CACHE["neff_cache"] = True


def kernel(x, qkv_w, proj_w, proj_b):
    from concourse.bass_utils import run_bass_kernel_spmd
    _install_neff_cache()

    f16 = np.float16
    x = np.asarray(x, dtype=np.float32)
    qkv_w = np.asarray(qkv_w, dtype=np.float32)
    proj_w = np.asarray(proj_w, dtype=np.float32)
    proj_b = np.asarray(proj_b, dtype=np.float32)

    # x^T rearranged to the SBUF image: [p, j*4096 + g*512 + c]
    xtb = (x.T.astype(f16).reshape(8, 128, 8, 512)
           .transpose(1, 2, 0, 3).reshape(128, 8 * 4096))
    xtb = np.ascontiguousarray(xtb)

    def w_img(w):  # [1024, 128] -> [p, g*128 + d]
        return np.ascontiguousarray(
            w.astype(f16).reshape(8, 128, 128).transpose(1, 0, 2)
            .reshape(128, 1024))

    in_maps = []
    for h in range(NCORES):
        in_maps.append({
            "xtb": xtb,
            "wqb": w_img(qkv_w[:, h * D:(h + 1) * D]),
            "wkb": w_img(qkv_w[:, G + h * D:G + (h + 1) * D]),
            "wvb": w_img(qkv_w[:, 2 * G + h * D:2 * G + (h + 1) * D]),
            "wpb": np.ascontiguousarray(
                proj_w[h * D:(h + 1) * D, :].astype(f16)),
        })

    nc = _get_nc()
    res = run_bass_kernel_spmd(nc, in_maps, list(range(NCORES)), trace=False)
    out = np.zeros((C, G), dtype=np.float32)
    for h in range(NCORES):
        out += res.results[h]["partial"].astype(np.float32)
    out += proj_b[None, :]
    return out


# revision 69
# speedup vs baseline: 1.0030x; 1.0030x over previous
"""Trainium2 Bass kernel for 8-head self-attention (nn_Attention2).

Sharding: one head per NeuronCore (tensor parallel over heads).
Each core computes, for its head h (d = 128 = partition width, C = 4096):
    K^T = Wk_h^T x^T           [d, C]
    Q^T = Wq_h^T x^T           [d, C]
    V   = x Wv_h               [C, d]   (128-row k-tiles)
    S^T tile = K_tile Q_chunk^T         (scores, key-major layout)
    P = exp(S^T / sqrt(d))              (fp16; no max-sub: |s~| < 6 here)
    O^T += V_tile^T P          [d, 512] per q-chunk, PSUM accum
    den[q] = P^T ones                   (1-column matmuls -> q-partition layout)
    partial = (O^T)^T Wp_h * (1/den)    (normalization fused into the
                                         projection-output copy as a
                                         per-partition scale)
The host sums the 8 per-head partials (tensor-parallel all-reduce) and adds
the bias.  All matmuls run in fp16 with fp32 PSUM accumulate.

Schedule: x^T streams in column-blocks of 512; per block PE computes the
K^T/Q^T/V contributions so compute starts ~3.5us into the DMA (warmup
matmuls arm the PE p-state ramp during the DMA window).  Score tiles for
q-chunk qc+1 are emitted during qc's flash-attention window so the
activation engine (the exp stream, ~133us) runs a full chunk ahead of the
PE's P*V consumption and never gates it.  Q^T blocks 2-7 and V blocks 6-7
are deferred into the windows as PE filler.

PSUM note: a matmul with start=True zeroes its whole 2KB bank ("zero
region") and only one accumulation group may be pending per region, so the
four per-window den columns share one group: only the first den matmul
starts it (the bank-zero clears all four columns), only the last stops it.
"""

import os

import numpy as np

C = 4096
G = 1024
D = 128
NCORES = 8
SCALE = float(D) ** -0.5
N_WARMUP = int(os.environ.get("N_WARMUP", "6"))

_CACHE = {}


def _build():
    import concourse.bacc as bacc
    import concourse.mybir as mybir
    from concourse.tile import TileContext

    F16 = mybir.dt.float16
    F32 = mybir.dt.float32
    Exp = mybir.ActivationFunctionType.Exp

    KC = G // 128   # 8 contraction chunks over the model dim
    NB = C // 512   # 8 column blocks of x^T (= q-chunks = key blocks)
    NQ = NB
    NT = C // 256   # 16 t-steps per q-chunk (2 key tiles each)

    nc = bacc.Bacc("TRN2", target_bir_lowering=False, debug=False,
                   num_devices=NCORES)
    # All inputs are host-prearranged SBUF images (partition-major).
    xtb_d = nc.dram_tensor("xtb", [128, NB * KC * 512], F16,
                           kind="ExternalInput").ap()
    wkb_d = nc.dram_tensor("wkb", [128, KC * D], F16, kind="ExternalInput").ap()
    wqb_d = nc.dram_tensor("wqb", [128, KC * D], F16, kind="ExternalInput").ap()
    wvb_d = nc.dram_tensor("wvb", [128, KC * D], F16, kind="ExternalInput").ap()
    wpb_d = nc.dram_tensor("wpb", [D, G], F16, kind="ExternalInput").ap()
    out_d = nc.dram_tensor("partial", [C, G], F16, kind="ExternalOutput").ap()

    with TileContext(nc) as tc:
        with (
            tc.tile_pool(name="persist", bufs=1) as big,
            tc.tile_pool(name="pt", bufs=20) as pt_pool,
            tc.tile_pool(name="recip", bufs=2) as recip_pool,
            tc.tile_pool(name="ob", bufs=4) as ob_pool,
            tc.tile_pool(name="ps_st", bufs=2, space="PSUM") as st_pool,
        ):
            xt_sb = big.tile([128, NB * KC * 512], F16)  # col j*4096+g*512+c
            wk_sb = big.tile([128, KC * D], F16)
            wq_sb = big.tile([128, KC * D], F16)
            wv_sb = big.tile([128, KC * D], F16)
            wp_sb = big.tile([128, G], F16)
            kt_sb = big.tile([128, C], F16)              # K^T
            qt_sb = big.tile([128, C], F16)              # Q^T
            v_sb = big.tile([128, C], F16)               # V, k-tile c at c*128
            ot_sb = big.tile([128, C], F16)              # O^T (unnormalized)
            ones_sb = big.tile([128, 1], F16)

            nc.vector.memset(ones_sb[:], 1.0)

            # ---- input DMA issues, split across three DGE paths ----
            # SP (HWDGE): the x^T block-0 quarters (the critical path).
            for q in range(4):
                nc.sync.dma_start(xt_sb[:, q * 1024:(q + 1) * 1024],
                                  xtb_d[:, q * 1024:(q + 1) * 1024])
            # Pool (SWDGE): wk halves, in parallel with SP's issues.
            nc.gpsimd.dma_start(wk_sb[:, 0:512], wkb_d[:, 0:512])
            nc.gpsimd.dma_start(wk_sb[:, 512:1024], wkb_d[:, 512:1024])
            # Act (HWDGE): wq halves, then wv early (so V-b0 can fill the
            # PE stall while block 1 lands), x^T block 1 quartered, then the
            # remaining blocks whole.
            nc.scalar.dma_start(wq_sb[:, 0:512], wqb_d[:, 0:512])
            nc.scalar.dma_start(wq_sb[:, 512:1024], wqb_d[:, 512:1024])
            nc.scalar.dma_start(wv_sb[:, 0:512], wvb_d[:, 0:512])
            nc.scalar.dma_start(wv_sb[:, 512:1024], wvb_d[:, 512:1024])
            for q in range(4):
                nc.scalar.dma_start(
                    xt_sb[:, 4096 + q * 1024:4096 + (q + 1) * 1024],
                    xtb_d[:, 4096 + q * 1024:4096 + (q + 1) * 1024])
            nc.scalar.dma_start(xt_sb[:, 2 * 4096:2 * 4096 + 2048],
                                xtb_d[:, 2 * 4096:2 * 4096 + 2048])
            nc.scalar.dma_start(xt_sb[:, 2 * 4096 + 2048:3 * 4096],
                                xtb_d[:, 2 * 4096 + 2048:3 * 4096])
            for j in range(3, NB):
                nc.scalar.dma_start(xt_sb[:, j * 4096:(j + 1) * 4096],
                                    xtb_d[:, j * 4096:(j + 1) * 4096])
            nc.scalar.dma_start(wp_sb[:], wpb_d[:, :])

            pts = {}

            def emit_st(qc, t):
                q_sl = qt_sb[:, qc * 512:(qc + 1) * 512]
                st = st_pool.tile([128, 1024], F32, tag="st", name="st")
                nc.tensor.matmul(st[:, 0:512],
                                 kt_sb[:, (2 * t) * 128:(2 * t + 1) * 128],
                                 q_sl, start=True, stop=True)
                nc.tensor.matmul(st[:, 512:1024],
                                 kt_sb[:, (2 * t + 1) * 128:(2 * t + 2) * 128],
                                 q_sl, start=True, stop=True)
                pt = pt_pool.tile([128, 1024], F16, tag="pt", name="pt")
                nc.scalar.activation(pt[:], st[:], Exp, scale=SCALE)
                pts[(qc, t)] = pt

            def emit_st_half(qh, t):
                # score step for a 256-query half-window: 4 k-tiles per tile
                # so the exp instruction stays at the efficient 1024 width.
                # Each 2KB zero region spans two 256-col segments: start on
                # the first, stop on the second.
                q_sl = qt_sb[:, qh * 256:(qh + 1) * 256]
                st = st_pool.tile([128, 1024], F32, tag="st", name="sth")
                for s in range(4):
                    kk = 4 * t + s
                    nc.tensor.matmul(st[:, s * 256:(s + 1) * 256],
                                     kt_sb[:, kk * 128:(kk + 1) * 128],
                                     q_sl, start=(s % 2 == 0),
                                     stop=(s % 2 == 1))
                pt = pt_pool.tile([128, 1024], F16, tag="pt", name="pth")
                nc.scalar.activation(pt[:], st[:], Exp, scale=SCALE)
                pts[("h", qh, t)] = pt

            def emit_qt_block(pool, j, tag="acc"):
                a = pool.tile([128, 512], F32, tag=tag, name="qta")
                for g in range(KC):
                    nc.tensor.matmul(
                        a[:], wq_sb[:, g * 128:(g + 1) * 128],
                        xt_sb[:, j * 4096 + g * 512:j * 4096 + (g + 1) * 512],
                        start=(g == 0), stop=(g == KC - 1))
                nc.vector.tensor_copy(qt_sb[:, j * 512:(j + 1) * 512], a[:])

            def emit_v_block(pool, j, tag="acc"):
                # g-outer so the first 16 matmuls need only wv's first half;
                # all four i-streams share one accumulation group (the first
                # matmul's start zeroes the whole bank, the last stops it)
                a = pool.tile([128, 512], F32, tag=tag, name="va")
                for g in range(KC):
                    for i in range(4):
                        nc.tensor.matmul(
                            a[:, i * 128:(i + 1) * 128],
                            xt_sb[:, j * 4096 + g * 512 + i * 128:
                                  j * 4096 + g * 512 + (i + 1) * 128],
                            wv_sb[:, g * 128:(g + 1) * 128],
                            start=(g == 0 and i == 0),
                            stop=(g == KC - 1 and i == 3))
                nc.vector.tensor_copy(v_sb[:, j * 512:(j + 1) * 512], a[:])

            # ---- phase 1: K^T (all), Q^T 0-1, V 0-5, qc0 scores ----
            with tc.tile_pool(name="ps_p1", bufs=4, space="PSUM") as ph1:
                # warmup: arm the PE p-state ramp while the first DMAs land
                # (rhs is a stride-0 broadcast of the ones column so the
                # chain only waits the tiny ones memset)
                warm_rhs = ones_sb[:, 0:1].broadcast_to([128, 512])
                wps = ph1.tile([128, 512], F32, tag="acc", name="wps")
                for w in range(N_WARMUP):
                    nc.tensor.matmul(wps[0:1, :], ones_sb[:, 0:1], warm_rhs,
                                     start=(w == 0), stop=(w == N_WARMUP - 1))

                # block 0: kt/qt interleaved per arriving x^T quarter
                ka = ph1.tile([128, 512], F32, tag="acc", name="kta")
                qa = ph1.tile([128, 512], F32, tag="acc", name="qta")
                for q in range(4):
                    for g in (2 * q, 2 * q + 1):
                        nc.tensor.matmul(
                            ka[:], wk_sb[:, g * 128:(g + 1) * 128],
                            xt_sb[:, g * 512:(g + 1) * 512],
                            start=(g == 0), stop=(g == KC - 1))
                    for g in (2 * q, 2 * q + 1):
                        nc.tensor.matmul(
                            qa[:], wq_sb[:, g * 128:(g + 1) * 128],
                            xt_sb[:, g * 512:(g + 1) * 512],
                            start=(g == 0), stop=(g == KC - 1))
                nc.vector.tensor_copy(kt_sb[:, 0:512], ka[:])
                nc.vector.tensor_copy(qt_sb[:, 0:512], qa[:])
                emit_st(0, 0)
                emit_st(0, 1)
                # V-b0 here: wv arrives before block 1, so this fills the
                # PE stall while block 1's quarters land
                emit_v_block(ph1, 0)

                for j in range(1, NB):
                    a = ph1.tile([128, 512], F32, tag="acc", name="kta")
                    for g in range(KC):
                        nc.tensor.matmul(
                            a[:], wk_sb[:, g * 128:(g + 1) * 128],
                            xt_sb[:, j * 4096 + g * 512:j * 4096 + (g + 1) * 512],
                            start=(g == 0), stop=(g == KC - 1))
                    nc.vector.tensor_copy(kt_sb[:, j * 512:(j + 1) * 512], a[:])
                    emit_st(0, 2 * j)
                    emit_st(0, 2 * j + 1)
                    if j == 1:
                        emit_qt_block(ph1, 1)
                    if 2 <= j < NB - 1:
                        # V lags kt by one block; the last V block (b5) lands
                        # at j=6 so its copy drains before the ph1 teardown
                        emit_v_block(ph1, j - 1)

            # ---- phase 2: attention windows + projection ----
            recips = {}
            with (
                tc.tile_pool(name="ps_o", bufs=3, space="PSUM") as o_pool,
                tc.tile_pool(name="ps_den", bufs=1, space="PSUM") as den_pool,
            ):
                # One den group per window in a single bank: the first matmul's
                # start=True zeroes the whole bank (all 4 columns); re-start in
                # the next window is WAR-ordered behind the reciprocal read.
                den_ps = den_pool.tile([128, 4], F32)

                def emit_proj(pqc, jj):
                    cq = pqc * 4 + jj
                    lhs = ot_sb[:, cq * 128:(cq + 1) * 128]
                    rs = recips[pqc][:, jj:jj + 1]
                    ob = ob_pool.tile([128, 1024], F16, name="ob")
                    pa = o_pool.tile([128, 512], F32, tag="o", name="pa")
                    nc.tensor.matmul(pa[:], lhs, wp_sb[:, 0:512],
                                     start=True, stop=True)
                    nc.vector.tensor_scalar_mul(ob[:, 0:512], pa[:], rs)
                    pb = o_pool.tile([128, 512], F32, tag="o", name="pb")
                    nc.tensor.matmul(pb[:], lhs, wp_sb[:, 512:1024],
                                     start=True, stop=True)
                    nc.vector.tensor_scalar_mul(ob[:, 512:1024], pb[:], rs)
                    nc.sync.dma_start(out_d[cq * 128:(cq + 1) * 128, :], ob[:])

                for qc in range(NQ - 1):
                    o_ps = o_pool.tile([128, 512], F32, tag="o", name="o_ps")
                    for t in range(NT):
                        if qc < NQ - 2:
                            emit_st(qc + 1, t)
                        elif t % 2 == 0:
                            # half-window A's scores (4 k-tiles per step)
                            emit_st_half(2 * NQ - 2, t // 2)
                        pt = pts.pop((qc, t))
                        nc.tensor.matmul(o_ps[:],
                                         v_sb[:, (2 * t) * 128:(2 * t + 1) * 128],
                                         pt[:, 0:512],
                                         start=(t == 0), stop=False)
                        nc.tensor.matmul(o_ps[:],
                                         v_sb[:, (2 * t + 1) * 128:(2 * t + 2) * 128],
                                         pt[:, 512:1024],
                                         start=False, stop=(t == NT - 1))
                        for hh in range(2):
                            for qq in range(4):
                                nc.tensor.matmul(
                                    den_ps[:, qq:qq + 1],
                                    pt[:, hh * 512 + qq * 128:
                                       hh * 512 + (qq + 1) * 128],
                                    ones_sb[:],
                                    start=(t == 0 and hh == 0 and qq == 0),
                                    stop=(t == NT - 1 and hh == 1 and qq == 3))
                        if qc > 0 and t % 4 == 2:
                            # one t-step before the multiple-of-4 boundary so
                            # the last pp-copy drains before the next window's
                            # o_ps allocation needs its slot
                            emit_proj(qc - 1, (t - 2) // 4)
                        # deferred phase-1 work as PE filler (PSUM slots
                        # borrowed from the o/pp pool)
                        if qc == 0:
                            if t == 4:
                                emit_v_block(o_pool, 6, tag="o")
                            elif t == 8:
                                emit_v_block(o_pool, 7, tag="o")
                            elif t == 12:
                                emit_qt_block(o_pool, 2, tag="o")
                        elif qc <= 5 and t == 8:
                            emit_qt_block(o_pool, qc + 2, tag="o")
                    recip = recip_pool.tile([128, 4], F32, name="recip")
                    nc.vector.reciprocal(recip[:], den_ps[:])
                    recips[qc] = recip
                    nc.vector.tensor_copy(
                        ot_sb[:, qc * 512:(qc + 1) * 512], o_ps[:])

                # ---- half-windows A and B (256 queries each) ----
                # A's projection and output DMA drain during B, so only two
                # row-tiles remain after the final P*V.
                def emit_proj_h(cq, rs, pa, pb, split=False, last=False):
                    lhs = ot_sb[:, cq * 128:(cq + 1) * 128]
                    ob = ob_pool.tile([128, 1024], F16, name="obh")
                    nc.tensor.matmul(pa, lhs, wp_sb[:, 0:512],
                                     start=True, stop=True)
                    nc.tensor.matmul(pb, lhs, wp_sb[:, 512:1024],
                                     start=True, stop=True)
                    if last:
                        # both halves on DVE (its queue flows without the
                        # cross-engine sem hop Act would pay) + per-half DMAs
                        nc.vector.tensor_scalar_mul(ob[:, 0:512], pa, rs)
                        nc.sync.dma_start(out_d[cq * 128:(cq + 1) * 128, 0:512],
                                          ob[:, 0:512])
                        nc.vector.tensor_scalar_mul(ob[:, 512:1024], pb, rs)
                        nc.sync.dma_start(
                            out_d[cq * 128:(cq + 1) * 128, 512:1024],
                            ob[:, 512:1024])
                    elif split:
                        # normalization copies split across DVE and Act so
                        # the endgame drains on two engines in parallel
                        nc.vector.tensor_scalar_mul(ob[:, 0:512], pa, rs)
                        nc.scalar.mul(ob[:, 512:1024], pb, rs)
                        nc.sync.dma_start(out_d[cq * 128:(cq + 1) * 128, :],
                                          ob[:])
                    else:
                        nc.vector.tensor_scalar_mul(ob[:, 0:512], pa, rs)
                        nc.vector.tensor_scalar_mul(ob[:, 512:1024], pb, rs)
                        nc.sync.dma_start(out_d[cq * 128:(cq + 1) * 128, :],
                                          ob[:])

                def emit_st_q(qh, t):
                    # quarter-window score step: 8 k-tiles x 128 queries
                    q_sl = qt_sb[:, qh * 128:(qh + 1) * 128]
                    st = st_pool.tile([128, 1024], F32, tag="st", name="stq")
                    for s in range(8):
                        kk = 8 * t + s
                        nc.tensor.matmul(st[:, s * 128:(s + 1) * 128],
                                         kt_sb[:, kk * 128:(kk + 1) * 128],
                                         q_sl, start=(s % 4 == 0),
                                         stop=(s % 4 == 3))
                    pt = pt_pool.tile([128, 1024], F16, tag="pt", name="ptq")
                    nc.scalar.activation(pt[:], st[:], Exp, scale=SCALE)
                    pts[("q", qh, t)] = pt

                # half-window A (queries 3584-3839): emits the first
                # quarter-window's scores and window 6's projection
                o_psA = o_pool.tile([128, 256], F32, tag="o", name="o_psA")
                for t in range(8):
                    if t % 2 == 0:
                        emit_st_q(2 * C // 256 - 2, t // 2)
                    pt = pts.pop(("h", 2 * NQ - 2, t))
                    for s in range(4):
                        kk = 4 * t + s
                        nc.tensor.matmul(
                            o_psA[:], v_sb[:, kk * 128:(kk + 1) * 128],
                            pt[:, s * 256:(s + 1) * 256],
                            start=(t == 0 and s == 0),
                            stop=(t == 7 and s == 3))
                        for qq in range(2):
                            nc.tensor.matmul(
                                den_ps[:, qq:qq + 1],
                                pt[:, s * 256 + qq * 128:
                                   s * 256 + (qq + 1) * 128],
                                ones_sb[:],
                                start=(t == 0 and s == 0 and qq == 0),
                                stop=(t == 7 and s == 3 and qq == 1))
                    if t in (1, 3, 5):
                        emit_proj(NQ - 2, (t - 1) // 2)
                    elif t == 6:
                        # last pair one step early so its copies drain before
                        # the A->B1 boundary needs the DVE
                        emit_proj(NQ - 2, 3)
                nc.vector.tensor_copy(
                    ot_sb[:, (NQ - 1) * 512:(NQ - 1) * 512 + 256], o_psA[:])
                recipA = recip_pool.tile([128, 2], F32, name="recipA")
                nc.vector.reciprocal(recipA[:], den_ps[:, 0:2])
                recips["A"] = recipA

                def quarter_window(qi, next_scores, projs):
                    # qi: quarter index 30/31 (128 queries); 4 steps of 8
                    # k-tiles; projs: list of (t, cq, recip_col_ap).
                    # den matmuls run one t-step late so the den-bank restart
                    # (WAR on the previous quarter's reciprocal) never stalls
                    # the PE.
                    o_psq = o_pool.tile([128, 128], F32, tag="o", name="o_psq")
                    live = {}

                    def emit_den(dt_, first, final):
                        pt = live[dt_]
                        for s in range(8):
                            nc.tensor.matmul(
                                den_ps[:, 0:1], pt[:, s * 128:(s + 1) * 128],
                                ones_sb[:],
                                start=(first and s == 0),
                                stop=(final and s == 7))

                    for t in range(4):
                        if next_scores:
                            emit_st_q(qi + 1, t)
                        pt = pts.pop(("q", qi, t))
                        live[t] = pt
                        for s in range(8):
                            kk = 8 * t + s
                            nc.tensor.matmul(
                                o_psq[:], v_sb[:, kk * 128:(kk + 1) * 128],
                                pt[:, s * 128:(s + 1) * 128],
                                start=(t == 0 and s == 0),
                                stop=(t == 3 and s == 7))
                        if t >= 2:
                            emit_den(t - 2, first=(t == 2), final=False)
                        for (pt_t, cq, rs) in projs:
                            if t == pt_t:
                                pa = o_pool.tile([128, 512], F32, tag="o",
                                                 name="pqa")
                                pb = o_pool.tile([128, 512], F32, tag="o",
                                                 name="pqb")
                                emit_proj_h(cq, rs, pa[:], pb[:], split=True)
                    emit_den(2, first=False, final=False)
                    emit_den(3, first=False, final=True)
                    return o_psq

                # quarter B1 (queries 3840-3967): emits B2's scores and A's
                # projection
                o_psB1 = quarter_window(
                    2 * C // 256 - 2, True,
                    [(1, (NQ - 1) * 4, recipA[:, 0:1]),
                     (2, (NQ - 1) * 4 + 1, recipA[:, 1:2])])
                nc.vector.tensor_copy(ot_sb[:, C - 256:C - 128], o_psB1[:])
                recipB1 = recip_pool.tile([128, 1], F32, name="recipB1")
                nc.vector.reciprocal(recipB1[:], den_ps[:, 0:1])

                # quarter B2 (queries 3968-4095): B1's projection inside
                o_psB2 = quarter_window(
                    2 * C // 256 - 1, False,
                    [(2, (NQ - 1) * 4 + 2, recipB1[:, 0:1])])
                nc.vector.tensor_copy(ot_sb[:, C - 128:C], o_psB2[:])
                recipB2 = recip_pool.tile([128, 1], F32, name="recipB2")
                nc.vector.reciprocal(recipB2[:], den_ps[:, 0:1])

                # ---- tail: only the final row-tile's projection remains ----
                pp = st_pool.tile([128, 1024], F32, tag="st", name="ppt")
                emit_proj_h((NQ - 1) * 4 + 3, recipB2[:, 0:1],
                            pp[:, 0:512], pp[:, 512:1024], last=True)

    nc.compile()
    return nc


def _get_nc():
    if "nc" not in _CACHE:
        _CACHE["nc"] = _build()
    return _CACHE["nc"]


def _install_neff_cache():
    """Content-hash cache for the walrus NEFF compile (~5 min saved on
    repeat runs of the same kernel)."""
    if _CACHE.get("neff_cache"):
        return
    import hashlib
    import shutil
    import concourse.bass_utils as bu
    import concourse.bass2jax as b2j

    orig = bu.compile_bir_kernel
    # The BIR embeds source paths/lines (debug info), so hashing it would
    # miss the cache when this file runs from a different directory. The
    # kernel is fully determined by this file's source, so key on that.
    with open(__file__, "rb") as f:
        src_hash = hashlib.sha256(f.read()).hexdigest()[:32]

    def cached_compile(bir_json, tmpdir, neff_name="file.neff"):
        key = src_hash
        cdir = os.path.expanduser("~/.cache/bass_neff")
        os.makedirs(cdir, exist_ok=True)
        cpath = os.path.join(cdir, key + ".neff")
        dst = os.path.join(tmpdir, neff_name)
        if os.path.exists(cpath):
            shutil.copy(cpath, dst)
            return dst
        out = orig(bir_json, tmpdir, neff_name)
        try:
            shutil.copy(out, cpath)
        except OSError:
            pass
        return out

    bu.compile_bir_kernel = cached_compile
    b2j.compile_bir_kernel = cached_compile
    _CACHE["neff_cache"] = True


def kernel(x, qkv_w, proj_w, proj_b):
    from concourse.bass_utils import run_bass_kernel_spmd
    _install_neff_cache()

    f16 = np.float16
    x = np.asarray(x, dtype=np.float32)
    qkv_w = np.asarray(qkv_w, dtype=np.float32)
    proj_w = np.asarray(proj_w, dtype=np.float32)
    proj_b = np.asarray(proj_b, dtype=np.float32)

    # x^T rearranged to the SBUF image: [p, j*4096 + g*512 + c]
    xtb = (x.T.astype(f16).reshape(8, 128, 8, 512)
           .transpose(1, 2, 0, 3).reshape(128, 8 * 4096))
    xtb = np.ascontiguousarray(xtb)

    def w_img(w):  # [1024, 128] -> [p, g*128 + d]
        return np.ascontiguousarray(
            w.astype(f16).reshape(8, 128, 128).transpose(1, 0, 2)
            .reshape(128, 1024))

    in_maps = []
    for h in range(NCORES):
        in_maps.append({
            "xtb": xtb,
            "wqb": w_img(qkv_w[:, h * D:(h + 1) * D]),
            "wkb": w_img(qkv_w[:, G + h * D:G + (h + 1) * D]),
            "wvb": w_img(qkv_w[:, 2 * G + h * D:2 * G + (h + 1) * D]),
            "wpb": np.ascontiguousarray(
                proj_w[h * D:(h + 1) * D, :].astype(f16)),
        })

    nc = _get_nc()
    res = run_bass_kernel_spmd(nc, in_maps, list(range(NCORES)), trace=False)
    out = np.zeros((C, G), dtype=np.float32)
    for h in range(NCORES):
        out += res.results[h]["partial"].astype(np.float32)
    out += proj_b[None, :]
    return out
